# revision 1
# baseline (speedup 1.0000x reference)
"""LocallyConnected2d (3x3, 64x64 out, C_in=16, C_out=32, B=32) on 8 trn2 cores.

out[b,o,h,w] = sum_{c,i,j} x[b,c,h+i,w+j] * weight[0,o,c,h,w,(i,j)] + bias[0,o,h,w]

Sharding: spatial over H_out - core i computes output rows 8i..8i+8, needing
input rows 8i..8i+10 (halo) and its 1/8 slice of the (per-position, unique)
weights. Weights dominate traffic (75MB total) and are read exactly once.

Math: per position (h,w) a K=145 contraction (9 taps x 16 ch + ones row
carrying the bias), M=32 (C_out, stationary), split K=128+17 with the K=17
part PSUM-accumulated right after (pairwise, so the accumulation dependency
is explicit in program order).

Quad packing: 4 adjacent positions share one matmul - lhsT [K, 4x32] and
rhs [K, 4x32] produce a [128, 128] PSUM block whose 32x32 diagonal blocks
are the 4 positions' [C_out, B] outputs (off-diagonal blocks are discarded).
This quarters the PE instruction count; the diagonal extraction is free:
the per-col-group out-DMA just slices partitions 32j..32j+32 x cols
32j..32j+32.

The host pre-replicates x into a [145, T, B] "shifted windows" tensor (one
16-channel band per kernel tap (i,j), shifted by 66*i+j, plus the ones row),
so the moving operand of quad (h, w0) is xr[:, 66h+w0 : 66h+w0+4, :] -
contiguous, no im2col on device.
"""

import numpy as np

import concourse.bass as bass
import concourse.mybir as mybir
import concourse.tile as tile
from concourse import bacc
from concourse import bass_utils

N_CORES = 8
B, CI, CO = 32, 16, 32
H = W = 64
HL = H // N_CORES          # output rows per core
XROWS = HL + 2             # input rows per core (with halo)
XW = 66
XFLAT = XROWS * XW         # 660
T = HL * XW                # 528: padded flat window (8 chunks of 66)
KA, KB = 128, 17           # contraction split
KT = KA + KB               # 145
NQ = W // 4                # 16 quads per row

USE_BF16 = True

_cache = {}


def _np_dt(use_bf16):
    if use_bf16:
        import ml_dtypes
        return np.dtype(ml_dtypes.bfloat16)
    return np.dtype(np.float32)


def _build(use_bf16, n_iters=1, mode="full"):
    assert mode in ("full", "lag", "dma", "xdma", "pe", "pelag")
    do_pe = mode not in ("dma", "xdma")
    pe_only = mode in ("pe", "pelag", "xdma")
    lag = mode in ("lag", "pelag")
    dt = mybir.dt.bfloat16 if use_bf16 else mybir.dt.float32
    f32 = mybir.dt.float32
    nc = bacc.Bacc("TRN2", target_bir_lowering=False, debug=False,
                   num_devices=N_CORES)
    xr_d = nc.dram_tensor("xr", [KT, T, B], dt, kind="ExternalInput")
    wsa_d = nc.dram_tensor("wsa", [HL, KA, W, CO], dt, kind="ExternalInput")
    wsb_d = nc.dram_tensor("wsb", [KB, HL, W, CO], dt, kind="ExternalInput")
    out_d = nc.dram_tensor("out", [HL, 4, CO, NQ, B], f32,
                           kind="ExternalOutput")

    import contextlib

    with tile.TileContext(nc) as tc:
        with (
            tc.tile_pool(name="px", bufs=1) as px,
            tc.tile_pool(name="pwa", bufs=4) as pwa,
            tc.tile_pool(name="pwb", bufs=2) as pwb,
            tc.tile_pool(name="po", bufs=2) as po,
            tc.tile_pool(name="pp", bufs=4, space=bass.MemorySpace.PSUM) as pp,
        ):
            loop = (tc.For_i(0, n_iters, 1) if n_iters > 1
                    else contextlib.nullcontext())
            with loop:
                pa = px.tile([KA, T, B], dt, tag="pa")
                pb = px.tile([KB, T, B], dt, tag="pb")
                for h in range(HL):
                    sl = slice(XW * h, XW * (h + 1))
                    nc.sync.dma_start(pa[:, sl, :], xr_d[0:KA, sl, :])
                nc.scalar.dma_start(pb[:], xr_d[KA:KT, :, :])

                wb_all = None
                if use_bf16:
                    wb_all = pwb.tile([KB, HL, W, CO], dt, tag="wb")
                    nc.scalar.dma_start(wb_all[:], wsb_d[:])

                if pe_only and mode != "xdma":
                    wa0 = pwa.tile([KA, W, CO], dt, tag="wa")
                    nc.scalar.dma_start(wa0[:], wsa_d[0])

                for h in range(HL):
                    weng = nc.sync if h % 2 == 0 else nc.scalar
                    stage = po.tile([KA, NQ, KA], f32, tag="stage")
                    if use_bf16:
                        wbh = wb_all[:, h] if wb_all is not None else None
                    else:
                        wbh = pwb.tile([KB, W, CO], dt, tag="wbh")
                        nc.scalar.dma_start(wbh[:], wsb_d[:, h])
                    if mode in ("dma", "xdma") and (h == 0 or mode == "dma"):
                        nc.gpsimd.memset(stage[:], 0.0)
                    if mode == "xdma":
                        wa = None
                    elif pe_only:
                        wa = wa0
                    else:
                        wa = pwa.tile([KA, W, CO], dt, tag="wa")
                        weng.dma_start(wa[:], wsa_d[h])
                    if do_pe and lag:
                        # lag-2 interleave: B(q) issues two MMs after A(q),
                        # hiding A's PSUM drain behind A(q+1)'s fill.
                        tiles, mma, mmb = [], [], []
                        for q in range(NQ):
                            w0 = 4 * q
                            t0 = XW * h + w0
                            ps = pp.tile([KA, KA], f32, tag="ps")
                            tiles.append(ps)
                            mma.append((ps, wa[:, w0:w0 + 4, :],
                                        pa[:, t0:t0 + 4, :]))
                            mmb.append((ps, wbh[:, w0:w0 + 4, :],
                                        pb[:, t0:t0 + 4, :]))
                        sched = []
                        for q in range(NQ):
                            sched.append(("a", q))
                            if q >= 1:
                                sched.append(("b", q - 1))
                        sched.append(("b", NQ - 1))
                        for kind, q in sched:
                            ps, lhs, rhs = mma[q] if kind == "a" else mmb[q]
                            nc.tensor.matmul(ps[:], lhs, rhs,
                                             start=(kind == "a"),
                                             stop=(kind == "b"))
                            if kind == "b":
                                nc.vector.tensor_copy(stage[:, q, :], ps[:])
                    elif do_pe:
                        # independent A/B quads in bank-grouped PSUM; two DVE
                        # passes (copy + accumulate) combine them.
                        for gq in range(NQ // 4):
                            psa = pp.tile([KA, 4, KA], f32, tag="psa")
                            psb = pp.tile([KA, 4, KA], f32, tag="psb")
                            for qq in range(4):
                                q = 4 * gq + qq
                                w0 = 4 * q
                                t0 = XW * h + w0
                                nc.tensor.matmul(psa[:, qq, :],
                                                 wa[:, w0:w0 + 4, :],
                                                 pa[:, t0:t0 + 4, :],
                                                 start=True, stop=True)
                            for qq in range(4):
                                q = 4 * gq + qq
                                w0 = 4 * q
                                t0 = XW * h + w0
                                nc.tensor.matmul(psb[:, qq, :],
                                                 wbh[:, w0:w0 + 4, :],
                                                 pb[:, t0:t0 + 4, :],
                                                 start=True, stop=True)
                            ssl = stage[:, 4 * gq:4 * (gq + 1), :]
                            nc.vector.tensor_copy(ssl, psa[:])
                            nc.vector.tensor_add(ssl, ssl, psb[:])
                    if not pe_only or h == 0:
                        for j in range(4):
                            nc.sync.dma_start(
                                out_d[h, j],
                                stage[32 * j:32 * (j + 1), :,
                                      32 * j:32 * (j + 1)])
    nc.compile()
    return nc


def _get_nc(use_bf16, n_iters=1, mode="full"):
    key = (use_bf16, n_iters, mode)
    if key not in _cache:
        _cache[key] = _build(use_bf16, n_iters, mode)
    return _cache[key]


def _pack_inputs(x, weight, bias, use_bf16):
    """Full inputs -> per-core in_maps (host-side shard + relayout)."""
    np_dt = _np_dt(use_bf16)
    x = np.asarray(x, np.float32)
    weight = np.asarray(weight, np.float32)
    bias = np.asarray(bias, np.float32)

    # weights: [1,o,c,h,w,k] -> [h, w, k=(tap,kc), o], bias appended as k=144
    wt = weight[0].transpose(2, 3, 4, 1, 0).reshape(H, W, 9 * CI, CO)
    bt = bias[0].transpose(1, 2, 0)[:, :, None, :]          # [h, w, 1, o]
    wfull = np.concatenate([wt, bt], axis=2)                # [h, w, 145, o]

    in_maps = []
    for c in range(N_CORES):
        r0 = HL * c
        xs = x[:, :, r0:r0 + XROWS, :].transpose(1, 0, 2, 3).reshape(
            CI, B, XFLAT)                                   # [c, b, flat]
        xr = np.zeros((KT, T, B), np.float32)
        for k in range(9):
            i, j = divmod(k, 3)
            off = XW * i + j
            blk = xs[:, :, off:off + T - 2]                 # [16, 32, 526]
            xr[16 * k:16 * (k + 1), :T - 2, :] = blk.transpose(0, 2, 1)
        xr[144, :, :] = 1.0
        in_maps.append({"xr": np.ascontiguousarray(xr, dtype=np_dt)})

        wc = wfull[r0:r0 + HL].transpose(0, 2, 1, 3)        # [h, k, w, o]
        in_maps[-1]["wsa"] = np.ascontiguousarray(wc[:, :KA], dtype=np_dt)
        in_maps[-1]["wsb"] = np.ascontiguousarray(
            wc[:, KA:].transpose(1, 0, 2, 3), dtype=np_dt)  # [kb, h, w, o]
    return in_maps


def _gather(results):
    # per-core out: [HL, 4(j), CO, NQ(q), B]; w = 4q + j
    outs = np.stack([results[c]["out"] for c in range(N_CORES)])
    out = outs.transpose(5, 3, 0, 1, 4, 2)     # [b, o, core, h, q, j]
    out = out.reshape(B, CO, H, W)
    return np.ascontiguousarray(out)


def run(x, weight, bias, use_bf16=None, n_iters=1, mode="full", **spmd_kwargs):
    if use_bf16 is None:
        use_bf16 = USE_BF16
    nc = _get_nc(use_bf16, n_iters, mode)
    in_maps = _pack_inputs(x, weight, bias, use_bf16)
    res = bass_utils.run_bass_kernel_spmd(nc, in_maps,
                                          core_ids=list(range(N_CORES)),
                                          **spmd_kwargs)
    return _gather(res.results), res


def kernel(x, weight, bias):
    out, _ = run(x, weight, bias)
    return out



# revision 8
# speedup vs baseline: 1.6763x; 1.6763x over previous
"""LocallyConnected2d (3x3, 64x64 out, C_in=16, C_out=32, B=32) on 8 trn2 cores.

out[b,o,h,w] = sum_{c,i,j} x[b,c,h+i,w+j] * weight[0,o,c,h,w,(i,j)] + bias[0,o,h,w]

Sharding: spatial over H_out - core i computes output rows 8i..8i+8, needing
input rows 8i..8i+10 (halo) and its 1/8 slice of the (per-position, unique)
weights.

Contraction K = 9 taps x 16 ch + ones row (bias) = 145, split as one K=128
matmul (A) + one K=17 matmul (B) that PSUM-accumulates onto A.

The host builds xa [128, 528, 32]: 8 "bands" of 16 channels, band for tap
(i,j) holding x shifted by 66i+j, plus a ones row at partition 127 (bias
rides matmul A through it).  The band ORDER is chosen so that matmul B's
operand is a +2-column-shifted view of partitions 0..17 of the SAME tile
(walrus requires fmap and weight to start at the same partition, so the
window sits at partition 0, like the wb weight tile):

  p0..15 = tap6 (i2,j0):  shifted +2 -> tap8 (i2,j2), all 16 ch
  p16    = tap0 (i0,j0) ch15:  shifted +2 -> tap2 ch15

so B covers exactly the 17 terms A is missing (tap2ch15 was evicted from A's
128 partitions to make room for the ones row; tap8 never fit).  No second
input tensor, no device-side bias add, no extra replication traffic.

Quad packing: 4 adjacent positions share one matmul - lhsT [K, 4x32] and
rhs [K, 4x32] produce a [128, 128] PSUM block whose 32x32 diagonal blocks
are the 4 positions' [C_out, B] outputs.  One PSUM tile [128, 16, 128]
(4 banks) holds a full output row; the diagonal is compacted to SBUF bf16
by 4 ops per row - in = ps[32j:32j+32, :, 32j:32j+32] - j 0,1 on DVE and
j 2,3 on ACT, so the output DMA is [128, 1KB] contiguous (1.05 MB total).

DMAs per core: SP ring carries xa column-chunks interleaved with 2-row wa
chunks; ACT ring carries 8 small per-row wb chunks then 8 per-row output
stores.  ~10.1 MB/core total traffic.
"""

import numpy as np

import concourse.bass as bass
import concourse.mybir as mybir
import concourse.tile as tile
from concourse import bacc
from concourse import bass_utils

N_CORES = 8
B, CI, CO = 32, 16, 32
H = W = 64
HL = H // N_CORES          # output rows per core: 8
XROWS = HL + 2             # input rows per core (halo): 10
XW = 66
XFLAT = XROWS * XW         # 660
T = HL * XW                # 528 padded flat window
NQ = W // 4                # 16 quads per row
KB = 17                    # second matmul contraction

# partition -> (tap t=3i+j, channel) for p0..126; p127 is the ones row
PARTS = ([(6, c) for c in range(16)] + [(0, 15)]
         + [(0, c) for c in range(15)]
         + [(1, c) for c in range(16)] + [(2, c) for c in range(15)]
         + [(3, c) for c in range(16)] + [(4, c) for c in range(16)]
         + [(5, c) for c in range(16)] + [(7, c) for c in range(16)])
assert len(PARTS) == 127
ONES_P = 127
BWIN = 0                   # matmul B reads xa[BWIN : BWIN+17] at col +2

_cache = {}


def _np_bf16():
    import ml_dtypes
    return np.dtype(ml_dtypes.bfloat16)


def _build(n_iters=1, mode="full"):
    assert mode in ("full", "dma", "pe")
    dt = mybir.dt.bfloat16
    f32 = mybir.dt.float32
    nc = bacc.Bacc("TRN2", target_bir_lowering=False, debug=False,
                   num_devices=N_CORES)
    xa_d = nc.dram_tensor("xa", [128, T, B], dt, kind="ExternalInput")
    wa_d = nc.dram_tensor("wa", [128, HL, W, CO], dt, kind="ExternalInput")
    wb_d = nc.dram_tensor("wb", [KB, HL, W, CO], dt, kind="ExternalInput")
    out_d = nc.dram_tensor("out", [HL, 128, NQ, 32], dt,
                           kind="ExternalOutput")

    import contextlib

    do_dma = mode in ("full", "dma")
    do_pe = mode in ("full", "pe")

    with tile.TileContext(nc) as tc:
        with (
            tc.tile_pool(name="px", bufs=1) as px,
            tc.tile_pool(name="pw", bufs=1) as pw,
            tc.tile_pool(name="po", bufs=3) as po,
            tc.tile_pool(name="pp", bufs=2, space=bass.MemorySpace.PSUM) as pp,
        ):
            loop = (tc.For_i(0, n_iters, 1) if n_iters > 1
                    else contextlib.nullcontext())
            with loop:
                xa = px.tile([128, T, B], dt, tag="xa")
                wa = pw.tile([128, HL, W, CO], dt, tag="wa")
                wb = pw.tile([KB, HL, W, CO], dt, tag="wb")

                # SP ring: per-row xa column-chunks interleaved with per-row
                # wa chunks, in row order, so row h unblocks early.
                if do_dma:
                    for k in range(HL):
                        nc.sync.dma_start(xa[:, XW * k:XW * (k + 1), :],
                                          xa_d[:, XW * k:XW * (k + 1), :])
                        nc.sync.dma_start(wa[:, k:k + 1], wa_d[:, k:k + 1])
                else:  # pe mode: minimal inputs
                    nc.sync.dma_start(xa[:, 0:XW, :], xa_d[:, 0:XW, :])
                    nc.sync.dma_start(wa[:, 0:1], wa_d[:, 0:1])
                # Pool/SWDGE: small per-row wb chunks (17 partitions each).
                for h in range(HL):
                    nc.gpsimd.dma_start(wb[:, h], wb_d[:, h])

                for h in range(HL):
                    stage = po.tile([128, NQ, 32], dt, tag="stage")
                    if mode == "dma" and h < 2:
                        nc.gpsimd.memset(stage[:], 0.0)
                    if do_pe:
                        ps = pp.tile([128, NQ, 128], f32, tag="ps")
                        for q in range(NQ):
                            w0 = 4 * q
                            t0 = XW * h + w0
                            nc.tensor.matmul(
                                ps[:, q, :],
                                wa[:, h, w0:w0 + 4, :],
                                xa[:, t0:t0 + 4, :],
                                start=True, stop=False)
                            nc.tensor.matmul(
                                ps[:, q, :],
                                wb[:, h, w0:w0 + 4, :],
                                xa[BWIN:BWIN + KB, t0 + 2:t0 + 6, :],
                                start=False, stop=True)
                        for j in range(4):
                            sj = slice(32 * j, 32 * (j + 1))
                            eng = nc.vector if j < 2 else nc.scalar
                            if j < 2:
                                eng.tensor_copy(stage[sj], ps[sj, :, sj])
                            else:
                                eng.copy(stage[sj], ps[sj, :, sj])
                    if do_dma or h == 0:
                        nc.scalar.dma_start(out_d[h], stage[:])
    nc.compile()
    return nc


def _get_nc(n_iters=1, mode="full"):
    key = (n_iters, mode)
    if key not in _cache:
        _cache[key] = _build(n_iters, mode)
    return _cache[key]


def _pack_inputs(x, weight, bias):
    """Full inputs -> per-core in_maps (host-side shard + relayout)."""
    bf16 = _np_bf16()
    x = np.asarray(x, np.float32)
    wt = np.asarray(weight, np.float32)[0]      # [o, c, h, w, 9]
    bt = np.asarray(bias, np.float32)[0]        # [o, h, w]

    in_maps = []
    for core in range(N_CORES):
        r0 = HL * core
        xs = x[:, :, r0:r0 + XROWS, :].transpose(1, 0, 2, 3).reshape(
            CI, B, XFLAT)                       # [c, b, flat]

        xa = np.zeros((128, T, B), np.float32)
        for p, (t, c) in enumerate(PARTS):
            i, j = divmod(t, 3)
            off = XW * i + j
            ncol = min(T, XFLAT - off)
            xa[p, :ncol, :] = xs[c, :, off:off + ncol].T
        xa[ONES_P, :, :] = 1.0
        in_maps.append({"xa": np.ascontiguousarray(xa, dtype=bf16)})

        # wa[p, h, w, o]
        wc = wt[:, :, r0:r0 + HL]               # [o, c, hl, w, 9]
        wa = np.zeros((128, HL, W, CO), np.float32)
        for p, (t, c) in enumerate(PARTS):
            wa[p] = wc[:, c, :, :, t].transpose(1, 2, 0)
        wa[ONES_P] = bt[:, r0:r0 + HL].transpose(1, 2, 0)
        in_maps[-1]["wa"] = np.ascontiguousarray(wa, dtype=bf16)

        wb = np.zeros((KB, HL, W, CO), np.float32)
        wb[:16] = wc[:, :, :, :, 8].transpose(1, 2, 3, 0)  # tap8 all ch
        wb[16] = wc[:, 15, :, :, 2].transpose(1, 2, 0)     # tap2 ch15
        in_maps[-1]["wb"] = np.ascontiguousarray(wb, dtype=bf16)
    return in_maps


def _gather(results):
    # per-core out: [HL, 128, NQ, 32] bf16, already diagonal-compacted:
    # p = 32*j + o, W = 4*q + j, H = 8*core + h.
    outs = np.stack([np.asarray(results[c]["out"], np.float32)
                     for c in range(N_CORES)])
    D = outs.reshape(N_CORES, HL, 4, 32, NQ, 32)   # [core, h, j, o, q, b]
    out = D.transpose(5, 3, 0, 1, 4, 2).reshape(B, CO, H, W)
    return np.ascontiguousarray(out)


def run(x, weight, bias, n_iters=1, mode="full", **spmd_kwargs):
    nc = _get_nc(n_iters, mode)
    in_maps = _pack_inputs(x, weight, bias)
    res = bass_utils.run_bass_kernel_spmd(nc, in_maps,
                                          core_ids=list(range(N_CORES)),
                                          **spmd_kwargs)
    return _gather(res.results), res


def kernel(x, weight, bias):
    out, _ = run(x, weight, bias)
    return out


# revision 32
# speedup vs baseline: 2.1573x; 1.2870x over previous
"""LocallyConnected2d (3x3, 64x64 out, C_in=16, C_out=32, B=32) on 8 trn2 cores.

out[b,o,h,w] = sum_{c,i,j} x[b,c,h+i,w+j] * weight[0,o,c,h,w,(i,j)] + bias[0,o,h,w]

Sharding: spatial over H_out - core i computes output rows 8i..8i+8, needing
input rows 8i..8i+10 (halo) and its 1/8 slice of the (per-position, unique)
weights.

Contraction K = 9 taps x 16 ch + ones row (bias) = 145, split as one K=128
matmul (A) + one K=32 matmul (B, 17 real rows zero-padded to 32 - measured
faster than K=17) that PSUM-accumulates onto A.  MMs are emitted in rounds
of A x4 (one per PSUM bank) then B x4: alternating the contraction size
every matmul measured ~270ns/switch on HW; round grouping cuts the number
of switches 4x while keeping the per-bank A(q)..B(q) order that PSUM's
bank-wide has_written clear requires.

The host builds xa [128, 528, 32]: 8 "bands" of 16 channels, band for tap
(i,j) holding x shifted by 66i+j, plus a ones row at partition 127 (bias
rides matmul A through it).  The band ORDER is chosen so that matmul B's
operand is a +2-column-shifted view of partitions 0..17 of the SAME tile
(walrus requires fmap and weight to start at the same partition, so the
window sits at partition 0, like the wb weight tile):

  p0..15 = tap6 (i2,j0):  shifted +2 -> tap8 (i2,j2), all 16 ch
  p16    = tap0 (i0,j0) ch15:  shifted +2 -> tap2 ch15

so B covers exactly the 17 terms A is missing (tap2ch15 was evicted from A's
128 partitions to make room for the ones row; tap8 never fit).  No second
input tensor, no device-side bias add, no extra replication traffic.

Quad packing: 4 adjacent positions share one matmul - lhsT [K, 4x32] and
rhs [K, 4x32] produce a [128, 128] PSUM block whose 32x32 diagonal blocks
are the 4 positions' [C_out, B] outputs.  One PSUM tile [128, 16, 128]
(4 banks) holds a full output row; the diagonal is compacted to SBUF bf16
by 4 ops per row - in = ps[32j:32j+32, :, 32j:32j+32] - j 0,1 on DVE and
j 2,3 on ACT, so the output DMA is [128, 1KB] contiguous (1.05 MB total).

DMAs per core: SP ring carries xa column-chunks interleaved with 2-row wa
chunks; ACT ring carries 8 small per-row wb chunks then 8 per-row output
stores.  ~10.1 MB/core total traffic.
"""

import numpy as np

import concourse.bass as bass
import concourse.mybir as mybir
import concourse.tile as tile
from concourse import bacc
from concourse import bass_utils

N_CORES = 8
B, CI, CO = 32, 16, 32
H = W = 64
HL = H // N_CORES          # output rows per core: 8
XROWS = HL + 2             # input rows per core (halo): 10
XW = 66
XFLAT = XROWS * XW         # 660
T = HL * XW                # 528 padded flat window
NQ = W // 4                # 16 quads per row
KB = 17                    # second matmul contraction

# partition -> (tap t=3i+j, channel) for p0..126; p127 is the ones row
PARTS = ([(6, c) for c in range(16)] + [(0, 15)]
         + [(0, c) for c in range(15)]
         + [(1, c) for c in range(16)] + [(2, c) for c in range(15)]
         + [(3, c) for c in range(16)] + [(4, c) for c in range(16)]
         + [(5, c) for c in range(16)] + [(7, c) for c in range(16)])
assert len(PARTS) == 127
ONES_P = 127
BWIN = 0                   # matmul B reads xa[BWIN : BWIN+17] at col +2

_cache = {}


def _np_bf16():
    import ml_dtypes
    return np.dtype(ml_dtypes.bfloat16)


CFG = dict(
    xa_chunks=4,        # xa split into this many column chunks
    wa_chunks=4,        # wa split into this many row chunks
    wb_chunks=8,        # wb split
    wb_eng="gpsimd",    # engine issuing wb loads
    out_eng="scalar",   # engine issuing output stores
    copy_split=2,       # first N of the 4 diagonal copies on DVE, rest ACT
    split_tiles=False,  # separate SBUF tile per input chunk (breaks WAW deps)
    in_engs=("sync",),  # engines cycled over input chunk DMAs
    qorder="bankrot",   # quad emission order: consecutive MM pairs hit
                        # different PSUM banks ("bankrot") or not ("linear")
    in_bufs=2,          # input tile-pool depth (2 = cross-iteration overlap)
    kbp=32,             # B-matmul contraction (17, or 32 = zero-padded)
    ab_round=True,      # emit A x4 (one per bank) then B x4, per round,
                        # reducing K-size alternations 4x (still correct:
                        # each bank sees A(q) ... B(q) before its next A)
    row_pair=False,     # interleave two rows (8 PSUM banks): A x8 / B x8
                        # per round - measured slower (PSUM group barrier)
)


def _build(n_iters=1, mode="full", **over):
    # aonly / aonly_samew / fatmm are PE-timing microbench modes (wrong math)
    assert mode in ("full", "dma", "pe", "nocopy", "noout",
                    "aonly", "aonly_samew", "fatmm")
    cfg = dict(CFG, **over)
    dt = mybir.dt.bfloat16
    f32 = mybir.dt.float32
    nc = bacc.Bacc("TRN2", target_bir_lowering=False, debug=False,
                   num_devices=N_CORES)
    xa_d = nc.dram_tensor("xa", [128, T, B], dt, kind="ExternalInput")
    wa_d = nc.dram_tensor("wa", [128, HL, W, CO], dt, kind="ExternalInput")
    wb_d = nc.dram_tensor("wb", [32, HL, W, CO], dt, kind="ExternalInput")
    kbp = cfg["kbp"]
    out_d = nc.dram_tensor("out", [HL, 128, NQ, 32], dt,
                           kind="ExternalOutput")

    import contextlib

    do_in_dma = mode != "pe"
    do_mm = mode != "dma"
    do_copy = mode in ("full", "noout", "pe")
    do_out = mode in ("full", "dma")
    micro = mode in ("aonly", "aonly_samew", "fatmm")

    with tile.TileContext(nc) as tc:
        with (
            tc.tile_pool(name="px", bufs=cfg["in_bufs"]) as px,
            tc.tile_pool(name="pw", bufs=cfg["in_bufs"]) as pw,
            tc.tile_pool(name="po", bufs=3) as po,
            tc.tile_pool(name="pp", bufs=2, space=bass.MemorySpace.PSUM) as pp,
        ):
            loop = (tc.For_i(0, n_iters, 1) if n_iters > 1
                    else contextlib.nullcontext())
            with loop:
                nx, nw = cfg["xa_chunks"], cfg["wa_chunks"]
                nwb = cfg["wb_chunks"]
                cx, cw, cwb = T // nx, HL // nw, HL // nwb
                in_engs = [getattr(nc, e) for e in cfg["in_engs"]]
                if cfg["split_tiles"]:
                    xat = [px.tile([128, cx, B], dt, tag=f"xa{k}",
                                   name=f"xa{k}") for k in range(nx)]
                    wat = [pw.tile([128, cw, W, CO], dt, tag=f"wa{k}",
                                   name=f"wa{k}") for k in range(nw)]
                    wbt = [pw.tile([kbp, cwb, W, CO], dt, tag=f"wb{k}",
                                   name=f"wb{k}") for k in range(nwb)]
                else:
                    xa = px.tile([128, T, B], dt, tag="xa")
                    wa = pw.tile([128, HL, W, CO], dt, tag="wa")
                    wb = pw.tile([kbp, HL, W, CO], dt, tag="wb")
                    xat = [xa[:, cx * k:cx * (k + 1), :] for k in range(nx)]
                    wat = [wa[:, cw * k:cw * (k + 1)] for k in range(nw)]
                    wbt = [wb[:, cwb * k:cwb * (k + 1)] for k in range(nwb)]

                # xa column chunks interleaved with wa row chunks, in row
                # order, so row h unblocks early.
                ei = 0
                if do_in_dma:
                    for k in range(max(nx, nw)):
                        if k < nx:
                            in_engs[ei % len(in_engs)].dma_start(
                                xat[k][:], xa_d[:, cx * k:cx * (k + 1), :])
                            ei += 1
                        if k < nw:
                            in_engs[ei % len(in_engs)].dma_start(
                                wat[k][:], wa_d[:, cw * k:cw * (k + 1)])
                            ei += 1
                else:  # pe mode: minimal inputs
                    nc.sync.dma_start(xat[0][:, 0:XW, :], xa_d[:, 0:XW, :])
                    nc.sync.dma_start(wat[0][:, 0:1], wa_d[:, 0:1])
                wbe = getattr(nc, cfg["wb_eng"])
                for k in range(nwb):
                    wbe.dma_start(wbt[k][:],
                                  wb_d[0:kbp, cwb * k:cwb * (k + 1)])

                oute = getattr(nc, cfg["out_eng"])
                rpg = 2 if (cfg["row_pair"] and do_mm and not micro) else 1
                for hg in range(0, HL, rpg):
                    ctx = []
                    for h in range(hg, hg + rpg):
                        stage = None
                        if do_copy or do_out:
                            stage = po.tile([128, NQ, 32], dt, tag="stage")
                        if mode == "dma":
                            nc.gpsimd.memset(stage[:], 0.0)
                        xac = xat[h // (cx // XW)]
                        xb = XW * (h % (cx // XW))
                        wac = wat[h // cw][:, h % cw]
                        wbc = wbt[h // cwb][:, h % cwb]
                        ps = None
                        if do_mm or micro:
                            ps = pp.tile([128, NQ, 128], f32, tag="ps")
                        ctx.append((h, stage, ps, xac, xb, wac, wbc))

                    if cfg["qorder"] == "bankrot":
                        qseq = [4 * (q % 4) + q // 4 for q in range(NQ)]
                    else:
                        qseq = list(range(NQ))

                    def mm_a(c, q):
                        _, _, ps, xac, xb, wac, _ = c
                        w0 = 4 * q
                        nc.tensor.matmul(
                            ps[:, q, :],
                            wac[:, w0:w0 + 4, :],
                            xac[:, xb + w0:xb + w0 + 4, :],
                            start=True, stop=False)

                    def mm_b(c, q):
                        _, _, ps, xac, xb, _, wbc = c
                        w0 = 4 * q
                        t0 = xb + w0
                        nc.tensor.matmul(
                            ps[:, q, :],
                            wbc[:, w0:w0 + 4, :],
                            xac[BWIN:BWIN + kbp, t0 + 2:t0 + 6, :],
                            start=False, stop=True)

                    if micro:
                        for c in ctx:
                            _, _, ps, xac, xb, wac, _ = c
                            if mode == "fatmm":
                                for k in range(4):
                                    nc.tensor.matmul(
                                        ps[:, 4 * k:4 * (k + 1), :],
                                        wac[:, 0:4, :],
                                        xac[:, xb + 16 * k:xb + 16 * k + 16, :],
                                        start=True, stop=True)
                            else:
                                for q in qseq:
                                    w0 = 0 if mode == "aonly_samew" else 4 * q
                                    t0 = xb + 4 * q
                                    nc.tensor.matmul(
                                        ps[:, q, :],
                                        wac[:, w0:w0 + 4, :],
                                        xac[:, t0:t0 + 4, :],
                                        start=True, stop=True)
                    elif do_mm:
                        if cfg["ab_round"]:
                            for r in range(4):
                                for c in ctx:
                                    for bk in range(4):
                                        mm_a(c, 4 * bk + r)
                                for c in ctx:
                                    for bk in range(4):
                                        mm_b(c, 4 * bk + r)
                        else:
                            for c in ctx:
                                for q in qseq:
                                    mm_a(c, q)
                                    mm_b(c, q)

                    for c in ctx:
                        h, stage, ps = c[0], c[1], c[2]
                        if do_copy:
                            for j in range(4):
                                sj = slice(32 * j, 32 * (j + 1))
                                if j < cfg["copy_split"]:
                                    nc.vector.tensor_copy(stage[sj],
                                                          ps[sj, :, sj])
                                else:
                                    nc.scalar.copy(stage[sj], ps[sj, :, sj])
                        if do_out or (do_copy and h == 0):
                            oute.dma_start(out_d[h], stage[:])
    nc.compile()
    return nc


def _get_nc(n_iters=1, mode="full", **over):
    key = (n_iters, mode, tuple(sorted(over.items())))
    if key not in _cache:
        _cache[key] = _build(n_iters, mode, **over)
    return _cache[key]


def _pack_inputs(x, weight, bias):
    """Full inputs -> per-core in_maps (host-side shard + relayout)."""
    bf16 = _np_bf16()
    x = np.asarray(x, np.float32)
    wt = np.asarray(weight, np.float32)[0]      # [o, c, h, w, 9]
    bt = np.asarray(bias, np.float32)[0]        # [o, h, w]

    in_maps = []
    for core in range(N_CORES):
        r0 = HL * core
        xs = x[:, :, r0:r0 + XROWS, :].transpose(1, 0, 2, 3).reshape(
            CI, B, XFLAT)                       # [c, b, flat]

        xa = np.zeros((128, T, B), np.float32)
        for p, (t, c) in enumerate(PARTS):
            i, j = divmod(t, 3)
            off = XW * i + j
            ncol = min(T, XFLAT - off)
            xa[p, :ncol, :] = xs[c, :, off:off + ncol].T
        xa[ONES_P, :, :] = 1.0
        in_maps.append({"xa": np.ascontiguousarray(xa, dtype=bf16)})

        # wa[p, h, w, o]
        wc = wt[:, :, r0:r0 + HL]               # [o, c, hl, w, 9]
        wa = np.zeros((128, HL, W, CO), np.float32)
        for p, (t, c) in enumerate(PARTS):
            wa[p] = wc[:, c, :, :, t].transpose(1, 2, 0)
        wa[ONES_P] = bt[:, r0:r0 + HL].transpose(1, 2, 0)
        in_maps[-1]["wa"] = np.ascontiguousarray(wa, dtype=bf16)

        # rows 17..31 stay zero: the kbp=32 variant streams real xa
        # partitions 17..31 against zero weights.
        wb = np.zeros((32, HL, W, CO), np.float32)
        wb[:16] = wc[:, :, :, :, 8].transpose(1, 2, 3, 0)  # tap8 all ch
        wb[16] = wc[:, 15, :, :, 2].transpose(1, 2, 0)     # tap2 ch15
        in_maps[-1]["wb"] = np.ascontiguousarray(wb, dtype=bf16)
    return in_maps


def _gather(results):
    # per-core out: [HL, 128, NQ, 32] bf16, already diagonal-compacted:
    # p = 32*j + o, W = 4*q + j, H = 8*core + h.
    outs = np.stack([np.asarray(results[c]["out"], np.float32)
                     for c in range(N_CORES)])
    D = outs.reshape(N_CORES, HL, 4, 32, NQ, 32)   # [core, h, j, o, q, b]
    out = D.transpose(5, 3, 0, 1, 4, 2).reshape(B, CO, H, W)
    return np.ascontiguousarray(out)


def run(x, weight, bias, n_iters=1, mode="full", **spmd_kwargs):
    nc = _get_nc(n_iters, mode)
    in_maps = _pack_inputs(x, weight, bias)
    res = bass_utils.run_bass_kernel_spmd(nc, in_maps,
                                          core_ids=list(range(N_CORES)),
                                          **spmd_kwargs)
    return _gather(res.results), res


def kernel(x, weight, bias):
    out, _ = run(x, weight, bias)
    return out
